# revision 35
# baseline (speedup 1.0000x reference)
"""ArcFace logits on 8 Trainium2 NeuronCores (Bass, raw engine streams).

out[n, c] = S * cos(theta_nc + M * [c == labels[n]]),  cos from L2-normalized
embeddings [1024, 512] x weight [100000, 512].

Model-parallel over the class dim (partial-FC): classes are padded/permuted
on the host so every core gets 12800 columns and its 128 label hits land on
the diagonal of output tile (chunk 0, row-block 0).  The compiled graph is
identical on all 8 cores and label-independent.

Final schedule (~230us/core, vs 320us fp32-I/O baseline; PE busy ~185us):
  - bf16 inputs / fp16 output, host-packed for 4KB+ contiguous descriptors
  - raw-e matmul; both norm scales fused into ONE DVE op per PSUM eviction:
        out = (psum * rsqe[row]) * rsqw[col]
  - PSUM arranged as 3 pair-banks of [128,1024]: one eviction instruction
    covers both chunks of a pair (104 evictions total, half the sem waits)
  - w column norms: DVE squares wt and pre-reduces over the 4 k-tiles
    (3 bf16 adds), so the PE ssq reduction is ONE [128x128]x[128x512]
    matmul per chunk with an all-ones lhsT (result pre-broadcast across
    partitions); ACT does exp(-0.5*ln(ssq)) straight out of PSUM
  - ssq/w-chain run two pairs ahead of consumption (NWT=8) so rsqw never
    gates an eviction
  - e row norms via ACT Square+accum_out on a row-major e copy, staged so
    the first evictions unblock early; ACT tables preloaded with a dummy op
  - the odd chunk (24) runs FIRST so the kernel tail is a full pair with
    overlapped eviction/DMA
"""

import math

import numpy as np
import ml_dtypes

import concourse.bass as bass
import concourse.mybir as mybir
from concourse.bass_utils import run_bass_kernel_spmd

AF = mybir.ActivationFunctionType
OP = mybir.AluOpType
F32 = mybir.dt.float32
F16 = mybir.dt.float16
BF16 = mybir.dt.bfloat16

S = 30.0
MARGIN = 0.5
N, D, C = 1024, 512, 100000

NCORES = 8
CS = 12800            # classes per core (padded: 8 * 12800 = 102400)
F = 512               # matmul free dim / class chunk width
NCHUNK = CS // F      # 25
KD = D // 128         # 4 contraction sub-tiles
NB = N // 128         # 8 row blocks
NWT = 8               # wt chunk buffers
NBCP = 4              # rsqw pair-buffer rotation
NOUTQ = 16            # out quad-buffer rotation
NPAIR = (NCHUNK - 1) // 2  # 12 pairs after the leading single chunk
NU = NB + NPAIR * NB  # 104 eviction units

COSM = float(math.cos(MARGIN))
SINM = float(math.sin(MARGIN))

# chunk processing order: odd chunk 24 first, then pairs (0,1),(2,3),...
SEQ = [24] + list(range(24))

# chunk-24 tile -> (pair-bank, half) map; tt 6,7 reuse banks 0,1 after tiles
# 0,2 evict, so pair-units 8,9,10 (banks 2,0,1) all satisfy the uniform
# "wait s_evu >= u-2" rule
C24_BANK = [0, 0, 1, 1, 2, 2, 0, 1]
C24_HALF = [0, 1, 0, 1, 0, 1, 0, 0]


def _mains_units_done(o):
    """s_mmu value once all main units of seq-chunk o are complete."""
    if o <= 0:
        return NB if o == 0 else 0
    return NB + NB * ((o - 1) // 2 + 1)


def _units_done_rwp(r):
    """s_evu value once all evictions using rwp index r are complete."""
    if r == 0:
        return NB
    return NB + NB * r


def build_graph():
    nc = bass.Bass(target_bir_lowering=False)

    eT_ext = nc.declare_dram_parameter("eT", [128, KD * N], BF16, isOutput=False)
    erow_ext = nc.declare_dram_parameter("erow", [128, NB * D], BF16, isOutput=False)
    w_ext = nc.declare_dram_parameter("w", [128, NCHUNK * KD * F], BF16, isOutput=False)
    ident_ext = nc.declare_dram_parameter("ident", [128, 128], F32, isOutput=False)
    onesm_ext = nc.declare_dram_parameter("onesm", [128, 128], BF16, isOutput=False)
    out_ext = nc.declare_dram_parameter("out", [N, CS], F16, isOutput=True)

    import contextlib

    ctx = contextlib.ExitStack()
    sb = lambda name, shape, dt=F32: ctx.enter_context(nc.sbuf_tensor(name, shape, dt))
    sem = lambda name: ctx.enter_context(nc.semaphore(name))

    with ctx:
        # --- SBUF ---
        eT_sb = sb("eT_sb", [128, KD * N], BF16)
        erow_sb = sb("erow_sb", [128, NB * D], BF16)
        wt = [sb(f"wt{b}", [128, KD * F], BF16) for b in range(NWT)]
        wsq_scr = sb("wsq_scr", [128, KD * F], BF16)
        wsum_hi = sb("wsum_hi", [128, F], BF16)
        wsum = [sb(f"wsum{b}", [128, F], BF16) for b in range(4)]
        sq_scr = sb("sq_scr", [128, D], BF16)
        esq_acc = sb("esq_acc", [128, NB])
        tmp8 = sb("tmp8", [128, NB])
        rsqe_sb = sb("rsqe_sb", [128, NB])
        lnw_tmp = sb("lnw_tmp", [128, F])
        rsqw_p = [sb(f"rsqw_p{b}", [128, 2 * F]) for b in range(NBCP)]
        outq = [sb(f"outq{b}", [128, 4 * F], F16) for b in range(NOUTQ)]
        outs = [sb(f"outs{b}", [128, F], F16) for b in range(NB)]
        ident_sb = sb("ident_sb", [128, 128])
        onesm_sb = sb("onesm_sb", [128, 128], BF16)
        diag_tmp = sb("diag_tmp", [128, 128])
        vdiag = sb("vdiag", [128, 1])
        sqv = sb("sqv", [128, 1])
        lnu = sb("lnu", [128, 1])
        s3v = sb("s3v", [128, 1])
        t1v = sb("t1v", [128, 1])
        fixp = sb("fixp", [128, 1])
        deltap = sb("deltap", [128, 1])
        lnS_b = sb("lnS_b", [128, 1])
        s2_b = sb("s2_b", [128, 1])

        # --- PSUM: 3 pair-banks [128,1024] + 2 ssq banks [128,512] = 16KB ---
        ps_pair = [
            ctx.enter_context(nc.psum_tensor(f"ps_pair{b}", [128, 2 * F], F32))
            for b in range(3)
        ]
        ps_ssq = [
            ctx.enter_context(nc.psum_tensor(f"ps_ssq{b}", [128, F], F32))
            for b in range(2)
        ]

        # --- semaphores ---
        s_const = sem("s_const")   # onesm
        s_ident = sem("s_ident")
        s_eT = sem("s_eT")
        s_erow = sem("s_erow")
        s_ms = sem("s_ms")
        s_wt = [sem(f"s_wt{b}") for b in range(NWT)]
        s_wsum = sem("s_wsum")     # DVE square+reduce done (seq-chunk count)
        s_ssqmm = sem("s_ssqmm")   # PE ssq matmul done
        s_lnw = sem("s_lnw")       # ACT Ln consumed ps_ssq
        s_rwp = sem("s_rwp")       # rsqw pair-buffer ready (chunk24=1, pair p=p+2)
        s_en = sem("s_en")         # rsqe ready (1: nb0, 2: nb1-3, 3: nb4-7)
        s_mmu = sem("s_mmu")       # PE unit done
        s_evu = sem("s_evu")       # DVE unit evicted
        s_vg = sem("s_vg")
        s_sfix = sem("s_sfix")
        s_vfix = sem("s_vfix")
        s_do = sem("s_do")         # quad out-DMA completions
        s_do24 = sem("s_do24")     # single (chunk 24) out-DMA completions

        with nc.Block() as block:

            @block.gpsimd
            def _(g):
                g.memset(lnS_b[:], float(np.log(S))).then_inc(s_ms, 1)
                g.memset(s2_b[:], float(S * S)).then_inc(s_ms, 1)

                def wt_dma(o):
                    c = SEQ[o]
                    g.dma_start(
                        out=wt[o % NWT][:],
                        in_=w_ext[:, c * KD * F:(c + 1) * KD * F],
                    ).then_inc(s_wt[o % NWT], 16)

                wt_dma(0)
                g.dma_start(out=onesm_sb[:], in_=onesm_ext[:]).then_inc(s_const, 16)
                # erow split: row-block 0 first so rsqe(0) unblocks early
                g.dma_start(out=erow_sb[:, 0:D], in_=erow_ext[:, 0:D]).then_inc(s_erow, 16)
                g.dma_start(out=eT_sb[:], in_=eT_ext[:]).then_inc(s_eT, 16)
                g.dma_start(out=erow_sb[:, D:NB * D],
                            in_=erow_ext[:, D:NB * D]).then_inc(s_erow, 16)
                g.dma_start(out=ident_sb[:], in_=ident_ext[:]).then_inc(s_ident, 16)
                for o in range(1, 7):
                    wt_dma(o)
                # chunk-24 singles as their evictions land
                for t in range(NB):
                    g.wait_ge(s_evu, t + 1)
                    g.dma_start(
                        out=out_ext[t * 128:(t + 1) * 128, 24 * F:25 * F],
                        in_=outs[t][:],
                    ).then_inc(s_do24, 16)
                for p in range(NPAIR):
                    for o in (2 * p + 7, 2 * p + 8):
                        if o <= NCHUNK - 1:
                            oo = o - NWT
                            if oo >= 0:
                                g.wait_ge(s_mmu, _mains_units_done(oo))
                            wt_dma(o)
                    if p % 2 == 1 and p < 10:
                        q = p // 2
                        for nb in range(NB):
                            g.wait_ge(s_evu, NB + NB * p + nb + 1)
                            if q == 0 and nb == 0:
                                g.wait_ge(s_vfix, 1)
                            qi = q * NB + nb
                            g.dma_start(
                                out=out_ext[nb * 128:(nb + 1) * 128,
                                            q * 4 * F:(q + 1) * 4 * F],
                                in_=outq[qi % NOUTQ][:],
                            ).then_inc(s_do, 16)
                    if p >= 10:
                        # last quad (q=5) split into pair-halves so the final
                        # post-eviction DMA burst is half-size and overlapped
                        h = p - 10
                        for nb in range(NB):
                            g.wait_ge(s_evu, NB + NB * p + nb + 1)
                            qi = 5 * NB + nb
                            g.dma_start(
                                out=out_ext[nb * 128:(nb + 1) * 128,
                                            5 * 4 * F + h * 2 * F:
                                            5 * 4 * F + (h + 1) * 2 * F],
                                in_=outq[qi % NOUTQ][:, h * 2 * F:(h + 1) * 2 * F],
                            ).then_inc(s_do, 16)
                g.wait_ge(s_do, 16 * ((NPAIR // 2 - 1) * NB + 2 * NB))
                g.wait_ge(s_do24, 16 * NB)

            @block.scalar
            def _(s):
                # dummy op: pulls the ACT table load off the critical path
                s.activation(sqv[:], vdiag[:], AF.Square)
                s.wait_ge(s_ms, 2)
                s.wait_ge(s_erow, 16)

                def esq_block(nb):
                    s.activation(
                        sq_scr[:], erow_sb[:, nb * D:(nb + 1) * D], AF.Square,
                        accum_out=esq_acc[:, nb:nb + 1],
                    )

                def rsqe_block(lo, hi):
                    s.drain()
                    s.activation(tmp8[:, lo:hi], esq_acc[:, lo:hi], AF.Ln)
                    s.drain()
                    s.activation(rsqe_sb[:, lo:hi], tmp8[:, lo:hi], AF.Exp,
                                 scale=-0.5, bias=lnS_b[:])
                    return s.drain()

                def w_chain(o, r, idx, last):
                    s.wait_ge(s_ssqmm, o + 1)
                    if r >= NBCP and idx == 0:
                        s.wait_ge(s_evu, _units_done_rwp(r - NBCP))
                    s.activation(lnw_tmp[:], ps_ssq[o % 2][:], AF.Ln).then_inc(s_lnw, 1)
                    s.drain()
                    s.activation(rsqw_p[r % NBCP][:, idx * F:(idx + 1) * F],
                                 lnw_tmp[:], AF.Exp, scale=-0.5)
                    d = s.drain()
                    if last:
                        d.then_inc(s_rwp, 1)

                # rsqe staged nb0 | w-chain(24) | nb1-3 | nb4-7
                esq_block(0)
                rsqe_block(0, 1).then_inc(s_en, 1)
                w_chain(0, 0, 0, True)
                s.wait_ge(s_erow, 32)
                for nb in range(1, 4):
                    esq_block(nb)
                rsqe_block(1, 4).then_inc(s_en, 1)
                for nb in range(4, NB):
                    esq_block(nb)
                rsqe_block(4, NB).then_inc(s_en, 1)
                for p in range(NPAIR):
                    w_chain(2 * p + 1, p + 1, 0, False)
                    w_chain(2 * p + 2, p + 1, 1, True)
                    if p == 0:
                        # margin trig on the gathered diagonal [128,1]
                        s.wait_ge(s_vg, 1)
                        s.activation(sqv[:], vdiag[:], AF.Square)
                        s.drain()
                        s.activation(lnu[:], sqv[:], AF.Ln, scale=-1.0, bias=s2_b[:])
                        s.drain()
                        s.activation(s3v[:], lnu[:], AF.Exp, scale=0.5)
                        s.activation(t1v[:], vdiag[:], AF.Copy, scale=COSM)
                        s.drain().then_inc(s_sfix, 1)

            @block.tensor
            def _(t):
                t.wait_ge(s_const, 16)  # onesm

                def do_ssq(o):
                    t.wait_ge(s_wsum, o + 1)
                    if o >= 2:
                        t.wait_ge(s_lnw, o - 1)  # ps_ssq[o%2] freed by Ln(o-2)
                    t.matmul(
                        ps_ssq[o % 2][:], lhsT=onesm_sb[:], rhs=wsum[o % 4][:],
                        start=True, stop=True, skip_group_check=True,
                    ).then_inc(s_ssqmm, 1)

                # ssq(0) first: it is ready (wsum chain) before eT posts, and
                # it starts the 4-hop rsqw chain that gates the first eviction
                do_ssq(0)
                t.wait_ge(s_eT, 16)
                # chunk 24: 8 single tiles in pair-bank halves.  Bank map is
                # chosen so every later pair-unit wait is uniformly u-2.
                for tt in range(NB):
                    if tt == 6:
                        t.wait_ge(s_evu, 1)
                    if tt == 7:
                        t.wait_ge(s_evu, 3)
                    bank = C24_BANK[tt]
                    half = C24_HALF[tt]
                    for k in range(KD):
                        mm = t.matmul(
                            ps_pair[bank][:, half * F:(half + 1) * F],
                            lhsT=eT_sb[:, k * N + tt * 128:k * N + (tt + 1) * 128],
                            rhs=wt[0][:, k * F:(k + 1) * F],
                            start=(k == 0), stop=(k == KD - 1),
                            skip_group_check=True,
                        )
                    mm.then_inc(s_mmu, 1)
                    # only the lnw-wait-free ssq(1) runs during warmup; the
                    # rest are spread over pairs 0-1 with the Ln pipe warm
                    if tt == 0:
                        do_ssq(1)
                # (pair, nb) slots for each remaining ssq o: transitional
                # schedule for o=2..8, then steady (2p+5)@nb1 / (2p+6)@nb5
                ssq_slots = {(0, 0): 2, (0, 2): 3, (0, 4): 4, (0, 6): 5,
                             (1, 0): 6, (1, 2): 7, (1, 5): 8}
                for p in range(2, NPAIR):
                    if 2 * p + 5 <= NCHUNK - 1:
                        ssq_slots[(p, 1)] = 2 * p + 5
                    if 2 * p + 6 <= NCHUNK - 1:
                        ssq_slots[(p, 5)] = 2 * p + 6
                for p in range(NPAIR):
                    o0, o1 = 2 * p + 1, 2 * p + 2
                    t.wait_ge(s_wt[o0 % NWT], 16 * (o0 // NWT + 1))
                    t.wait_ge(s_wt[o1 % NWT], 16 * (o1 // NWT + 1))
                    for nb in range(NB):
                        u = NB + NB * p + nb
                        t.wait_ge(s_evu, u - 2)
                        for k in range(KD):
                            for j, ob in ((0, o0), (1, o1)):
                                mm = t.matmul(
                                    ps_pair[u % 3][:, j * F:(j + 1) * F],
                                    lhsT=eT_sb[:, k * N + nb * 128:k * N + (nb + 1) * 128],
                                    rhs=wt[ob % NWT][:, k * F:(k + 1) * F],
                                    start=(k == 0), stop=(k == KD - 1),
                                    skip_group_check=True,
                                )
                        mm.then_inc(s_mmu, 1)
                        if (p, nb) in ssq_slots:
                            do_ssq(ssq_slots[(p, nb)])

            @block.vector
            def _(v):
                def prep(o):
                    v.wait_ge(s_wt[o % NWT], 16 * (o // NWT + 1))
                    if o >= 4:
                        v.wait_ge(s_ssqmm, o - 3)  # wsum[o%4] freed by ssq(o-4)
                    b = o % NWT
                    v.tensor_tensor(out=wsq_scr[:], in0=wt[b][:], in1=wt[b][:],
                                    op=OP.mult)
                    v.tensor_tensor(out=wsum[o % 4][:], in0=wsq_scr[:, 0:F],
                                    in1=wsq_scr[:, F:2 * F], op=OP.add)
                    v.tensor_tensor(out=wsum_hi[:], in0=wsq_scr[:, 2 * F:3 * F],
                                    in1=wsq_scr[:, 3 * F:4 * F], op=OP.add)
                    v.tensor_tensor(out=wsum[o % 4][:], in0=wsum[o % 4][:],
                                    in1=wsum_hi[:], op=OP.add).then_inc(s_wsum, 1)

                prep(0)
                prep(1)
                prep(2)
                # chunk 24 evictions (units 0..7)
                v.wait_ge(s_en, 1)
                v.wait_ge(s_rwp, 1)
                for tt in range(NB):
                    if tt == 1:
                        v.wait_ge(s_en, 2)
                    if tt == 4:
                        v.wait_ge(s_en, 3)
                    bank = C24_BANK[tt]
                    half = C24_HALF[tt]
                    v.wait_ge(s_mmu, tt + 1)
                    v.scalar_tensor_tensor(
                        outs[tt][:], ps_pair[bank][:, half * F:(half + 1) * F],
                        rsqe_sb[:, tt:tt + 1], rsqw_p[0][:, 0:F],
                        OP.mult, OP.mult,
                    ).then_inc(s_evu, 1)
                    if tt == 2:
                        prep(3)
                    if tt == 5:
                        prep(4)
                for p in range(NPAIR):
                    q, h = p // 2, p % 2
                    r = p + 1
                    for nb in range(NB):
                        if nb == 0:
                            if 2 * p + 5 <= NCHUNK - 1:
                                prep(2 * p + 5)
                            v.wait_ge(s_rwp, r + 1)
                        if nb == 4 and 2 * p + 6 <= NCHUNK - 1:
                            prep(2 * p + 6)
                        u = NB + NB * p + nb
                        qi = q * NB + nb
                        if qi >= NOUTQ and h == 0:
                            v.wait_ge(s_do, 16 * (qi - NOUTQ + 1))
                        v.wait_ge(s_mmu, u + 1)
                        v.scalar_tensor_tensor(
                            outq[qi % NOUTQ][:, h * 2 * F:(h + 1) * 2 * F],
                            ps_pair[u % 3][:], rsqe_sb[:, nb:nb + 1],
                            rsqw_p[r % NBCP][:], OP.mult, OP.mult,
                        ).then_inc(s_evu, 1)
                        if p == 0 and nb == 0:
                            # extract the label diagonal of tile (0,0)
                            v.wait_ge(s_ident, 16)
                            v.drain()
                            v.tensor_tensor(out=diag_tmp[:], in0=outq[0][:, 0:128],
                                            in1=ident_sb[:], op=OP.mult)
                            v.drain()
                            v.tensor_reduce(vdiag[:], diag_tmp[:],
                                            mybir.AxisListType.X, OP.add)
                            v.drain().then_inc(s_vg, 1)
                    if p == 0:
                        # margin rewrite after pair-0 evictions
                        v.wait_ge(s_sfix, 1)
                        v.scalar_tensor_tensor(fixp[:], s3v[:], -SINM, t1v[:],
                                               OP.mult, OP.add)
                        v.drain()
                        v.tensor_tensor(out=deltap[:], in0=fixp[:], in1=vdiag[:],
                                        op=OP.subtract)
                        v.drain()
                        v.scalar_tensor_tensor(outq[0][:, 0:128], ident_sb[:],
                                               deltap[:], outq[0][:, 0:128],
                                               OP.mult, OP.add)
                        v.drain().then_inc(s_vfix, 1)

    return nc


_GRAPH = None


def _get_graph():
    global _GRAPH
    if _GRAPH is None:
        _GRAPH = build_graph()
    return _GRAPH


def _host_prepare(embeddings, weight, labels):
    """Row/class permutations putting each core's labels on the (0,0) diagonal,
    packed into partition-contiguous bf16 layouts."""
    labels = np.asarray(labels).astype(np.int64)
    e = np.asarray(embeddings, dtype=np.float32)
    w = np.asarray(weight, dtype=np.float32)

    first_seen = {}
    extras = []  # (core, col, row, cls) for duplicate label classes
    for i in range(N):
        l = int(labels[i])
        m, p = i // 128, i % 128
        if l not in first_seen:
            first_seen[l] = (m, p)
        else:
            extras.append((m, p, i, l))

    labeled = np.zeros(C, dtype=bool)
    labeled[labels] = True
    unlab = np.nonzero(~labeled)[0]

    colmaps = np.full((NCORES, CS), -1, dtype=np.int64)
    for i in range(N):
        colmaps[i // 128, i % 128] = labels[i]
    fill_slots = NCORES * (CS - 128)
    fill = np.full(fill_slots, -1, dtype=np.int64)
    fill[: unlab.size] = unlab
    fill = fill.reshape(NCORES, CS - 128)
    colmaps[:, 128:] = fill

    valid_bulk = colmaps >= 0
    for (m, p, i, l) in extras:
        valid_bulk[m, p] = False

    e_bf = e.astype(ml_dtypes.bfloat16)
    wTfull = w.T  # [512, 100000] view
    ident = np.eye(128, dtype=np.float32)
    onesm = np.ones((128, 128), dtype=ml_dtypes.bfloat16)
    in_maps = []
    row_perms = []
    for m in range(NCORES):
        cm = colmaps[m]
        wsel = np.zeros((D, CS), dtype=np.float32)
        vmask = cm >= 0
        wsel[:, vmask] = wTfull[:, cm[vmask]]
        wsel[0, ~vmask] = 1.0
        w_bf = wsel.astype(ml_dtypes.bfloat16)
        # [D=(ko,p), CS=(c,f)] -> [p, c, ko, f]
        wprep = np.ascontiguousarray(
            w_bf.reshape(KD, 128, NCHUNK, F).transpose(1, 2, 0, 3)
        ).reshape(128, NCHUNK * KD * F)
        rows = np.concatenate([
            np.arange(m * 128, (m + 1) * 128),
            np.delete(np.arange(N), np.s_[m * 128:(m + 1) * 128]),
        ])
        row_perms.append(rows)
        e_perm = e_bf[rows]                      # [N, D]
        eTprep = np.ascontiguousarray(
            e_perm.T.reshape(KD, 128, N).transpose(1, 0, 2)
        ).reshape(128, KD * N)
        erow = np.ascontiguousarray(
            e_perm.reshape(NB, 128, D).transpose(1, 0, 2)
        ).reshape(128, NB * D)
        in_maps.append({
            "eT": eTprep,
            "erow": erow,
            "w": wprep,
            "ident": ident,
            "onesm": onesm,
        })
    return in_maps, row_perms, colmaps, valid_bulk, extras


def _assemble(results, row_perms, colmaps, valid_bulk, extras):
    out = np.empty((N, C), dtype=np.float32)
    slabs = []
    for m in range(NCORES):
        slab = results[m]["out"].astype(np.float32)
        unperm = np.empty_like(slab)
        unperm[row_perms[m]] = slab
        slabs.append(unperm)
        vb = valid_bulk[m]
        out[:, colmaps[m][vb]] = unperm[:, vb]
    for (m, p, i, l) in extras:
        out[i, l] = slabs[m][i, p]
    return out


def kernel(embeddings, weight, labels, _trace=False):
    nc = _get_graph()
    in_maps, row_perms, colmaps, valid_bulk, extras = _host_prepare(
        embeddings, weight, labels
    )
    res = run_bass_kernel_spmd(nc, in_maps, core_ids=list(range(NCORES)), trace=_trace)
    out = _assemble(res.results, row_perms, colmaps, valid_bulk, extras)
    if _trace:
        return out, res
    return out


# revision 39
# speedup vs baseline: 1.0397x; 1.0397x over previous
"""ArcFace logits on 8 Trainium2 NeuronCores (Bass, raw engine streams).

out[n, c] = S * cos(theta_nc + M * [c == labels[n]]),  cos from L2-normalized
embeddings [1024, 512] x weight [100000, 512].

Model-parallel over the class dim (partial-FC): classes are padded/permuted
on the host so every core gets 12800 columns and its 128 label hits land on
the diagonal of output tile (chunk 0, row-block 0).  The compiled graph is
identical on all 8 cores and label-independent.

Final schedule (~230us/core, vs 320us fp32-I/O baseline; PE busy ~185us):
  - bf16 inputs / fp16 output, host-packed for 4KB+ contiguous descriptors
  - raw-e matmul; both norm scales fused into ONE DVE op per PSUM eviction:
        out = (psum * rsqe[row]) * rsqw[col]
  - PSUM arranged as 3 pair-banks of [128,1024]: one eviction instruction
    covers both chunks of a pair (104 evictions total, half the sem waits)
  - w column norms: DVE squares wt and pre-reduces over the 4 k-tiles
    (3 bf16 adds), so the PE ssq reduction is ONE [128x128]x[128x512]
    matmul per chunk with an all-ones lhsT (result pre-broadcast across
    partitions); ACT does exp(-0.5*ln(ssq)) straight out of PSUM
  - ssq/w-chain run two pairs ahead of consumption (NWT=8) so rsqw never
    gates an eviction
  - e row norms via ACT Square+accum_out on a row-major e copy, staged so
    the first evictions unblock early; ACT tables preloaded with a dummy op
  - the odd chunk (24) runs FIRST so the kernel tail is a full pair with
    overlapped eviction/DMA
"""

import math

import numpy as np
import ml_dtypes

import concourse.bass as bass
import concourse.mybir as mybir
from concourse.bass_utils import run_bass_kernel_spmd

AF = mybir.ActivationFunctionType
OP = mybir.AluOpType
F32 = mybir.dt.float32
F16 = mybir.dt.float16
BF16 = mybir.dt.bfloat16

S = 30.0
MARGIN = 0.5
N, D, C = 1024, 512, 100000

NCORES = 8
CS = 12800            # classes per core (padded: 8 * 12800 = 102400)
F = 512               # matmul free dim / class chunk width
NCHUNK = CS // F      # 25
KD = D // 128         # 4 contraction sub-tiles
NB = N // 128         # 8 row blocks
NWT = 8               # wt chunk buffers
NBCP = 4              # rsqw pair-buffer rotation
NOUTQ = 16            # out quad-buffer rotation
NPAIR = (NCHUNK - 1) // 2  # 12 pairs after the leading single chunk
NU = NB + NPAIR * NB  # 104 eviction units

COSM = float(math.cos(MARGIN))
SINM = float(math.sin(MARGIN))

# chunk processing order: odd chunk 24 first, then pairs (0,1),(2,3),...
SEQ = [24] + list(range(24))

# chunk-24 tile -> (pair-bank, half) map; tt 6,7 reuse banks 0,1 after tiles
# 0,2 evict, so pair-units 8,9,10 (banks 2,0,1) all satisfy the uniform
# "wait s_evu >= u-2" rule
C24_BANK = [0, 0, 1, 1, 2, 2, 0, 1]
C24_HALF = [0, 1, 0, 1, 0, 1, 0, 0]


def _mains_units_done(o):
    """s_mmu value once all main units of seq-chunk o are complete."""
    if o <= 0:
        return NB if o == 0 else 0
    return NB + NB * ((o - 1) // 2 + 1)


def _units_done_rwp(r):
    """s_evu value once all evictions using rwp index r are complete."""
    if r == 0:
        return NB
    return NB + NB * r


def build_graph():
    nc = bass.Bass(target_bir_lowering=False)

    eT_ext = nc.declare_dram_parameter("eT", [128, KD * N], BF16, isOutput=False)
    erow_ext = nc.declare_dram_parameter("erow", [128, NB * D], BF16, isOutput=False)
    w_ext = nc.declare_dram_parameter("w", [128, NCHUNK * KD * F], BF16, isOutput=False)
    ident_ext = nc.declare_dram_parameter("ident", [128, 128], F32, isOutput=False)
    onesm_ext = nc.declare_dram_parameter("onesm", [128, 128], BF16, isOutput=False)
    out_ext = nc.declare_dram_parameter("out", [N, CS], F16, isOutput=True)

    import contextlib

    ctx = contextlib.ExitStack()
    sb = lambda name, shape, dt=F32: ctx.enter_context(nc.sbuf_tensor(name, shape, dt))
    sem = lambda name: ctx.enter_context(nc.semaphore(name))

    with ctx:
        # --- SBUF ---
        eT_sb = sb("eT_sb", [128, KD * N], BF16)
        erow_sb = sb("erow_sb", [128, NB * D], BF16)
        wt = [sb(f"wt{b}", [128, KD * F], BF16) for b in range(NWT)]
        wsq_scr = sb("wsq_scr", [128, KD * F], BF16)
        wsum_hi = sb("wsum_hi", [128, F], BF16)
        wsum = [sb(f"wsum{b}", [128, F], BF16) for b in range(4)]
        sq_scr = sb("sq_scr", [128, D], BF16)
        esq_acc = sb("esq_acc", [128, NB])
        tmp8 = sb("tmp8", [128, NB])
        rsqe_sb = sb("rsqe_sb", [128, NB])
        lnw_tmp = sb("lnw_tmp", [128, F])
        rsqw_p = [sb(f"rsqw_p{b}", [128, 2 * F]) for b in range(NBCP)]
        outq = [sb(f"outq{b}", [128, 4 * F], F16) for b in range(NOUTQ)]
        outs = [sb(f"outs{b}", [128, F], F16) for b in range(NB)]
        ident_sb = sb("ident_sb", [128, 128])
        onesm_sb = sb("onesm_sb", [128, 128], BF16)
        diag_tmp = sb("diag_tmp", [128, 128])
        vdiag = sb("vdiag", [128, 1])
        sqv = sb("sqv", [128, 1])
        lnu = sb("lnu", [128, 1])
        s3v = sb("s3v", [128, 1])
        t1v = sb("t1v", [128, 1])
        fixp = sb("fixp", [128, 1])
        deltap = sb("deltap", [128, 1])
        lnS_b = sb("lnS_b", [128, 1])
        s2_b = sb("s2_b", [128, 1])

        # --- PSUM: 3 pair-banks [128,1024] + 2 ssq banks [128,512] = 16KB ---
        ps_pair = [
            ctx.enter_context(nc.psum_tensor(f"ps_pair{b}", [128, 2 * F], F32))
            for b in range(3)
        ]
        ps_ssq = [
            ctx.enter_context(nc.psum_tensor(f"ps_ssq{b}", [128, F], F32))
            for b in range(2)
        ]

        # --- semaphores ---
        s_const = sem("s_const")   # onesm
        s_ident = sem("s_ident")
        s_eT = sem("s_eT")
        s_erow = sem("s_erow")
        s_ms = sem("s_ms")
        s_wt = [sem(f"s_wt{b}") for b in range(NWT)]
        s_wsum = sem("s_wsum")     # DVE square+reduce done (seq-chunk count)
        s_ssqmm = sem("s_ssqmm")   # PE ssq matmul done
        s_lnw = sem("s_lnw")       # ACT Ln consumed ps_ssq
        s_rwp = sem("s_rwp")       # rsqw pair-buffer ready (chunk24=1, pair p=p+2)
        s_en = sem("s_en")         # rsqe ready (1: nb0, 2: nb1-3, 3: nb4-7)
        s_mmu = sem("s_mmu")       # PE unit done
        s_evu = sem("s_evu")       # DVE unit evicted
        s_vg = sem("s_vg")
        s_sfix = sem("s_sfix")
        s_vfix = sem("s_vfix")
        s_do = sem("s_do")         # quad out-DMA completions
        s_do24 = sem("s_do24")     # single (chunk 24) out-DMA completions

        with nc.Block() as block:

            @block.gpsimd
            def _(g):
                g.memset(lnS_b[:], float(np.log(S))).then_inc(s_ms, 1)
                g.memset(s2_b[:], float(S * S)).then_inc(s_ms, 1)

                def wt_dma(o):
                    c = SEQ[o]
                    g.dma_start(
                        out=wt[o % NWT][:],
                        in_=w_ext[:, c * KD * F:(c + 1) * KD * F],
                    ).then_inc(s_wt[o % NWT], 16)

                wt_dma(0)
                g.dma_start(out=onesm_sb[:], in_=onesm_ext[:]).then_inc(s_const, 16)
                # erow split: row-block 0 first so rsqe(0) unblocks early
                g.dma_start(out=erow_sb[:, 0:D], in_=erow_ext[:, 0:D]).then_inc(s_erow, 16)
                g.dma_start(out=eT_sb[:], in_=eT_ext[:]).then_inc(s_eT, 16)
                g.dma_start(out=erow_sb[:, D:NB * D],
                            in_=erow_ext[:, D:NB * D]).then_inc(s_erow, 16)
                g.dma_start(out=ident_sb[:], in_=ident_ext[:]).then_inc(s_ident, 16)
                for o in range(1, 7):
                    wt_dma(o)
                # chunk-24 singles as their evictions land
                for t in range(NB):
                    g.wait_ge(s_evu, t + 1)
                    g.dma_start(
                        out=out_ext[t * 128:(t + 1) * 128, 24 * F:25 * F],
                        in_=outs[t][:],
                    ).then_inc(s_do24, 16)
                for p in range(NPAIR):
                    for o in (2 * p + 7, 2 * p + 8):
                        if o <= NCHUNK - 1:
                            oo = o - NWT
                            if oo >= 0:
                                g.wait_ge(s_mmu, _mains_units_done(oo))
                            wt_dma(o)
                    if p % 2 == 1 and p < 10:
                        q = p // 2
                        for nb in range(NB):
                            g.wait_ge(s_evu, NB + NB * p + nb + 1)
                            if q == 0 and nb == 0:
                                g.wait_ge(s_vfix, 1)
                            qi = q * NB + nb
                            g.dma_start(
                                out=out_ext[nb * 128:(nb + 1) * 128,
                                            q * 4 * F:(q + 1) * 4 * F],
                                in_=outq[qi % NOUTQ][:],
                            ).then_inc(s_do, 16)
                    if p >= 10:
                        # last quad (q=5) split into pair-halves so the final
                        # post-eviction DMA burst is half-size and overlapped
                        h = p - 10
                        for nb in range(NB):
                            g.wait_ge(s_evu, NB + NB * p + nb + 1)
                            qi = 5 * NB + nb
                            g.dma_start(
                                out=out_ext[nb * 128:(nb + 1) * 128,
                                            5 * 4 * F + h * 2 * F:
                                            5 * 4 * F + (h + 1) * 2 * F],
                                in_=outq[qi % NOUTQ][:, h * 2 * F:(h + 1) * 2 * F],
                            ).then_inc(s_do, 16)
                g.wait_ge(s_do, 16 * ((NPAIR // 2 - 1) * NB + 2 * NB))
                g.wait_ge(s_do24, 16 * NB)

            @block.scalar
            def _(s):
                # dummy op: pulls the ACT table load off the critical path
                s.activation(sqv[:], vdiag[:], AF.Square)
                s.wait_ge(s_ms, 2)
                s.wait_ge(s_erow, 16)

                def esq_block(nb):
                    s.activation(
                        sq_scr[:], erow_sb[:, nb * D:(nb + 1) * D], AF.Square,
                        accum_out=esq_acc[:, nb:nb + 1],
                    )

                def rsqe_block(lo, hi):
                    s.drain()
                    s.activation(tmp8[:, lo:hi], esq_acc[:, lo:hi], AF.Ln)
                    s.drain()
                    s.activation(rsqe_sb[:, lo:hi], tmp8[:, lo:hi], AF.Exp,
                                 scale=-0.5, bias=lnS_b[:])
                    return s.drain()

                def w_chain(o, r, idx, last):
                    s.wait_ge(s_ssqmm, o + 1)
                    if r >= NBCP and idx == 0:
                        s.wait_ge(s_evu, _units_done_rwp(r - NBCP))
                    s.activation(lnw_tmp[:], ps_ssq[o % 2][:], AF.Ln).then_inc(s_lnw, 1)
                    s.drain()
                    s.activation(rsqw_p[r % NBCP][:, idx * F:(idx + 1) * F],
                                 lnw_tmp[:], AF.Exp, scale=-0.5)
                    d = s.drain()
                    if last:
                        d.then_inc(s_rwp, 1)

                # rsqe staged nb0 | w-chain(24) | nb1-3 | nb4-7
                esq_block(0)
                rsqe_block(0, 1).then_inc(s_en, 1)
                w_chain(0, 0, 0, True)
                s.wait_ge(s_erow, 32)
                for nb in range(1, 4):
                    esq_block(nb)
                rsqe_block(1, 4).then_inc(s_en, 1)
                for nb in range(4, NB):
                    esq_block(nb)
                rsqe_block(4, NB).then_inc(s_en, 1)
                for p in range(NPAIR):
                    w_chain(2 * p + 1, p + 1, 0, False)
                    w_chain(2 * p + 2, p + 1, 1, True)
                    if p == 0:
                        # margin trig on the gathered diagonal [128,1]
                        s.wait_ge(s_vg, 1)
                        s.activation(sqv[:], vdiag[:], AF.Square)
                        s.drain()
                        s.activation(lnu[:], sqv[:], AF.Ln, scale=-1.0, bias=s2_b[:])
                        s.drain()
                        s.activation(s3v[:], lnu[:], AF.Exp, scale=0.5)
                        s.activation(t1v[:], vdiag[:], AF.Copy, scale=COSM)
                        s.drain().then_inc(s_sfix, 1)

            @block.tensor
            def _(t):
                t.wait_ge(s_const, 16)  # onesm

                def do_ssq(o):
                    t.wait_ge(s_wsum, o + 1)
                    if o >= 2:
                        t.wait_ge(s_lnw, o - 1)  # ps_ssq[o%2] freed by Ln(o-2)
                    t.matmul(
                        ps_ssq[o % 2][:], lhsT=onesm_sb[:], rhs=wsum[o % 4][:],
                        start=True, stop=True, skip_group_check=True,
                    ).then_inc(s_ssqmm, 1)

                # ssq(0) first: it is ready (wsum chain) before eT posts, and
                # it starts the 4-hop rsqw chain that gates the first eviction
                do_ssq(0)
                t.wait_ge(s_eT, 16)
                # chunk 24: 8 single tiles in pair-bank halves.  Bank map is
                # chosen so every later pair-unit wait is uniformly u-2.
                for tt in range(NB):
                    if tt == 6:
                        t.wait_ge(s_evu, 1)
                    if tt == 7:
                        t.wait_ge(s_evu, 3)
                    bank = C24_BANK[tt]
                    half = C24_HALF[tt]
                    for k in range(KD):
                        mm = t.matmul(
                            ps_pair[bank][:, half * F:(half + 1) * F],
                            lhsT=eT_sb[:, k * N + tt * 128:k * N + (tt + 1) * 128],
                            rhs=wt[0][:, k * F:(k + 1) * F],
                            start=(k == 0), stop=(k == KD - 1),
                            skip_group_check=True,
                        )
                    mm.then_inc(s_mmu, 1)
                    # only the lnw-wait-free ssq(1) runs during warmup; the
                    # rest are spread over pairs 0-1 with the Ln pipe warm
                    if tt == 0:
                        do_ssq(1)
                # (pair, nb) slots for each remaining ssq o; a full pair
                # behind the DVE square/adds that produce its wsum, so the
                # s_wsum wait never stalls
                ssq_slots = {(0, 0): 2, (0, 1): 3, (0, 5): 4}
                for p in range(1, NPAIR):
                    if 2 * p + 3 <= NCHUNK - 1:
                        ssq_slots[(p, 1)] = 2 * p + 3
                    if 2 * p + 4 <= NCHUNK - 1:
                        ssq_slots[(p, 5)] = 2 * p + 4
                for p in range(NPAIR):
                    o0, o1 = 2 * p + 1, 2 * p + 2
                    t.wait_ge(s_wt[o0 % NWT], 16 * (o0 // NWT + 1))
                    t.wait_ge(s_wt[o1 % NWT], 16 * (o1 // NWT + 1))
                    for nb in range(NB):
                        u = NB + NB * p + nb
                        t.wait_ge(s_evu, u - 2)
                        for k in range(KD):
                            for j, ob in ((0, o0), (1, o1)):
                                mm = t.matmul(
                                    ps_pair[u % 3][:, j * F:(j + 1) * F],
                                    lhsT=eT_sb[:, k * N + nb * 128:k * N + (nb + 1) * 128],
                                    rhs=wt[ob % NWT][:, k * F:(k + 1) * F],
                                    start=(k == 0), stop=(k == KD - 1),
                                    skip_group_check=True,
                                )
                        mm.then_inc(s_mmu, 1)
                        if (p, nb) in ssq_slots:
                            do_ssq(ssq_slots[(p, nb)])

            @block.vector
            def _(v):
                def prep_sq(o):
                    v.wait_ge(s_wt[o % NWT], 16 * (o // NWT + 1))
                    b = o % NWT
                    v.tensor_tensor(out=wsq_scr[:], in0=wt[b][:], in1=wt[b][:],
                                    op=OP.mult)

                def prep_adds(o):
                    if o >= 4:
                        v.wait_ge(s_ssqmm, o - 3)  # wsum[o%4] freed by ssq(o-4)
                    v.tensor_tensor(out=wsum[o % 4][:], in0=wsq_scr[:, 0:F],
                                    in1=wsq_scr[:, F:2 * F], op=OP.add)
                    v.tensor_tensor(out=wsum_hi[:], in0=wsq_scr[:, 2 * F:3 * F],
                                    in1=wsq_scr[:, 3 * F:4 * F], op=OP.add)
                    v.tensor_tensor(out=wsum[o % 4][:], in0=wsum[o % 4][:],
                                    in1=wsum_hi[:], op=OP.add).then_inc(s_wsum, 1)

                def prep(o):
                    prep_sq(o)
                    prep_adds(o)

                prep(0)
                prep(1)
                prep(2)
                # chunk 24 evictions (units 0..7)
                v.wait_ge(s_en, 1)
                v.wait_ge(s_rwp, 1)
                for tt in range(NB):
                    if tt == 1:
                        v.wait_ge(s_en, 2)
                    if tt == 4:
                        v.wait_ge(s_en, 3)
                    bank = C24_BANK[tt]
                    half = C24_HALF[tt]
                    v.wait_ge(s_mmu, tt + 1)
                    v.scalar_tensor_tensor(
                        outs[tt][:], ps_pair[bank][:, half * F:(half + 1) * F],
                        rsqe_sb[:, tt:tt + 1], rsqw_p[0][:, 0:F],
                        OP.mult, OP.mult,
                    ).then_inc(s_evu, 1)
                    # square and adds in separate slots: the eviction stream
                    # never pauses longer than one op
                    if tt == 2:
                        prep_sq(3)
                    if tt == 3:
                        prep_adds(3)
                    if tt == 4:
                        prep_sq(4)
                    if tt == 5:
                        prep_adds(4)
                for p in range(NPAIR):
                    q, h = p // 2, p % 2
                    r = p + 1
                    for nb in range(NB):
                        if nb == 0:
                            if 2 * p + 5 <= NCHUNK - 1:
                                prep_sq(2 * p + 5)
                            v.wait_ge(s_rwp, r + 1)
                        if nb == 1 and 2 * p + 5 <= NCHUNK - 1:
                            prep_adds(2 * p + 5)
                        if nb == 4 and 2 * p + 6 <= NCHUNK - 1:
                            prep_sq(2 * p + 6)
                        if nb == 5 and 2 * p + 6 <= NCHUNK - 1:
                            prep_adds(2 * p + 6)
                        u = NB + NB * p + nb
                        qi = q * NB + nb
                        if qi >= NOUTQ and h == 0:
                            v.wait_ge(s_do, 16 * (qi - NOUTQ + 1))
                        v.wait_ge(s_mmu, u + 1)
                        v.scalar_tensor_tensor(
                            outq[qi % NOUTQ][:, h * 2 * F:(h + 1) * 2 * F],
                            ps_pair[u % 3][:], rsqe_sb[:, nb:nb + 1],
                            rsqw_p[r % NBCP][:], OP.mult, OP.mult,
                        ).then_inc(s_evu, 1)
                        if p == 0 and nb == 0:
                            # extract the label diagonal of tile (0,0)
                            v.wait_ge(s_ident, 16)
                            v.drain()
                            v.tensor_tensor(out=diag_tmp[:], in0=outq[0][:, 0:128],
                                            in1=ident_sb[:], op=OP.mult)
                            v.drain()
                            v.tensor_reduce(vdiag[:], diag_tmp[:],
                                            mybir.AxisListType.X, OP.add)
                            v.drain().then_inc(s_vg, 1)
                    if p == 0:
                        # margin rewrite after pair-0 evictions
                        v.wait_ge(s_sfix, 1)
                        v.scalar_tensor_tensor(fixp[:], s3v[:], -SINM, t1v[:],
                                               OP.mult, OP.add)
                        v.drain()
                        v.tensor_tensor(out=deltap[:], in0=fixp[:], in1=vdiag[:],
                                        op=OP.subtract)
                        v.drain()
                        v.scalar_tensor_tensor(outq[0][:, 0:128], ident_sb[:],
                                               deltap[:], outq[0][:, 0:128],
                                               OP.mult, OP.add)
                        v.drain().then_inc(s_vfix, 1)

    return nc


_GRAPH = None


def _get_graph():
    global _GRAPH
    if _GRAPH is None:
        _GRAPH = build_graph()
    return _GRAPH


def _host_prepare(embeddings, weight, labels):
    """Row/class permutations putting each core's labels on the (0,0) diagonal,
    packed into partition-contiguous bf16 layouts."""
    labels = np.asarray(labels).astype(np.int64)
    e = np.asarray(embeddings, dtype=np.float32)
    w = np.asarray(weight, dtype=np.float32)

    first_seen = {}
    extras = []  # (core, col, row, cls) for duplicate label classes
    for i in range(N):
        l = int(labels[i])
        m, p = i // 128, i % 128
        if l not in first_seen:
            first_seen[l] = (m, p)
        else:
            extras.append((m, p, i, l))

    labeled = np.zeros(C, dtype=bool)
    labeled[labels] = True
    unlab = np.nonzero(~labeled)[0]

    colmaps = np.full((NCORES, CS), -1, dtype=np.int64)
    for i in range(N):
        colmaps[i // 128, i % 128] = labels[i]
    fill_slots = NCORES * (CS - 128)
    fill = np.full(fill_slots, -1, dtype=np.int64)
    fill[: unlab.size] = unlab
    fill = fill.reshape(NCORES, CS - 128)
    colmaps[:, 128:] = fill

    valid_bulk = colmaps >= 0
    for (m, p, i, l) in extras:
        valid_bulk[m, p] = False

    e_bf = e.astype(ml_dtypes.bfloat16)
    wTfull = w.T  # [512, 100000] view
    ident = np.eye(128, dtype=np.float32)
    onesm = np.ones((128, 128), dtype=ml_dtypes.bfloat16)
    in_maps = []
    row_perms = []
    for m in range(NCORES):
        cm = colmaps[m]
        wsel = np.zeros((D, CS), dtype=np.float32)
        vmask = cm >= 0
        wsel[:, vmask] = wTfull[:, cm[vmask]]
        wsel[0, ~vmask] = 1.0
        w_bf = wsel.astype(ml_dtypes.bfloat16)
        # [D=(ko,p), CS=(c,f)] -> [p, c, ko, f]
        wprep = np.ascontiguousarray(
            w_bf.reshape(KD, 128, NCHUNK, F).transpose(1, 2, 0, 3)
        ).reshape(128, NCHUNK * KD * F)
        rows = np.concatenate([
            np.arange(m * 128, (m + 1) * 128),
            np.delete(np.arange(N), np.s_[m * 128:(m + 1) * 128]),
        ])
        row_perms.append(rows)
        e_perm = e_bf[rows]                      # [N, D]
        eTprep = np.ascontiguousarray(
            e_perm.T.reshape(KD, 128, N).transpose(1, 0, 2)
        ).reshape(128, KD * N)
        erow = np.ascontiguousarray(
            e_perm.reshape(NB, 128, D).transpose(1, 0, 2)
        ).reshape(128, NB * D)
        in_maps.append({
            "eT": eTprep,
            "erow": erow,
            "w": wprep,
            "ident": ident,
            "onesm": onesm,
        })
    return in_maps, row_perms, colmaps, valid_bulk, extras


def _assemble(results, row_perms, colmaps, valid_bulk, extras):
    out = np.empty((N, C), dtype=np.float32)
    slabs = []
    for m in range(NCORES):
        slab = results[m]["out"].astype(np.float32)
        unperm = np.empty_like(slab)
        unperm[row_perms[m]] = slab
        slabs.append(unperm)
        vb = valid_bulk[m]
        out[:, colmaps[m][vb]] = unperm[:, vb]
    for (m, p, i, l) in extras:
        out[i, l] = slabs[m][i, p]
    return out


def kernel(embeddings, weight, labels, _trace=False):
    nc = _get_graph()
    in_maps, row_perms, colmaps, valid_bulk, extras = _host_prepare(
        embeddings, weight, labels
    )
    res = run_bass_kernel_spmd(nc, in_maps, core_ids=list(range(NCORES)), trace=_trace)
    out = _assemble(res.results, row_perms, colmaps, valid_bulk, extras)
    if _trace:
        return out, res
    return out
